# revision 2
# baseline (speedup 1.0000x reference)
"""GumbelVectorQuantizer eval-path kernel for 8 Trainium2 NeuronCores (Bass).

Sharding: data-parallel over BT rows (32768 -> 4096/core); projection W
[768,640], bias, and codebook [640,128] replicated. Per core, per
128-row tile:

  PE   : logits = x @ W + b computed exactly via an fp32r hi/lo split
         (x = xr + xl, W = wr + wl, all fp32r-rounded on host;
         logits = xr@wr + xl@wr + xr@wl, error ~2e-6 << the 1.88e-4
         min top-2 argmax gap), plus a rank-1 bias matmul that opens
         each PSUM accumulation group.
  ACT  : PSUM->SBUF logits copy, exp(lg-max) with accumulated row sum,
         ln, normalized-softmax exp pass.
  DVE  : row max (negated), top-8 + argmax indices.
  PE   : per-group ones^T @ softmax accumulated over all tiles -> [1,320]
         probability partials for the perplexity term.
  DMA  : indirect gather of codebook rows by argmax index -> q.

Host: shards/transposes/splits inputs, sums the [2,320] partials across
cores in float64 and computes quantize_prob_ppl there (cheaper than an
all-reduce for 2.5 KB). Falls back to a jax.pmap implementation if the
Bass stack is unavailable.
"""

import functools

import numpy as np

GROUPS = 2
NUM_VARS = 320
VAR_DIM = 128
CURR_TEMP = 2.0
EPS = 1e-7
N_CORES = 8

P = 128
FEAT = 768
KCH = FEAT // P  # 6
G = GROUPS
V = NUM_VARS
D = VAR_DIM
ROWS = 4096  # per-core row shard (32768 / 8)


def _round_fp32r(a: np.ndarray) -> np.ndarray:
    """Round fp32 to fp32r (11 explicit mantissa bits), round-to-nearest-even."""
    u = a.astype(np.float32).view(np.uint32).astype(np.uint64)
    drop = 12
    half = np.uint64(1 << (drop - 1))
    lsb = (u >> np.uint64(drop)) & np.uint64(1)
    u = u + half - np.uint64(1) + lsb
    u = (u >> np.uint64(drop)) << np.uint64(drop)
    return u.astype(np.uint32).view(np.float32)


def _build(nc, rows):
    import concourse.mybir as mybir
    import concourse.tile as tile
    from concourse.bass import IndirectOffsetOnAxis

    F32 = mybir.dt.float32
    F32R = mybir.dt.float32r
    AF = mybir.ActivationFunctionType
    ntiles = rows // P

    xr_t = nc.dram_tensor("xr_t", [FEAT, rows], F32, kind="ExternalInput").ap()
    xl_t = nc.dram_tensor("xl_t", [FEAT, rows], F32, kind="ExternalInput").ap()
    wr = nc.dram_tensor("wr", [FEAT, G * V], F32, kind="ExternalInput").ap()
    wl = nc.dram_tensor("wl", [FEAT, G * V], F32, kind="ExternalInput").ap()
    bpack = nc.dram_tensor("bpack", [1, P + G * V], F32, kind="ExternalInput").ap()
    cb = nc.dram_tensor("cb", [G * V, D], F32, kind="ExternalInput").ap()
    q = nc.dram_tensor("q", [rows, G * D], F32, kind="ExternalOutput").ap()
    pp = nc.dram_tensor("pp", [G, V], F32, kind="ExternalOutput").ap()

    with tile.TileContext(nc) as tc:
        with (
            tc.tile_pool(name="const", bufs=1) as cpool,
            tc.tile_pool(name="xin", bufs=3) as xpool,
            tc.tile_pool(name="work", bufs=3) as wkpool,
            tc.tile_pool(name="small", bufs=4) as smpool,
            tc.tile_pool(name="psum", bufs=2, space="PSUM") as pspool,
            tc.tile_pool(name="ppsum", bufs=1, space="PSUM") as pppool,
        ):
            w_stage = cpool.tile([P, KCH, G * V], F32, name="w_stage")
            wr_sb = cpool.tile([P, KCH, G * V], F32R, name="wr_sb")
            wl_sb = cpool.tile([P, KCH, G * V], F32R, name="wl_sb")
            nc.sync.dma_start(out=w_stage[:], in_=wr.rearrange("(c p) n -> p c n", p=P))
            nc.scalar.activation(out=wr_sb[:], in_=w_stage[:], func=AF.Copy)
            w_stage2 = cpool.tile([P, KCH, G * V], F32, name="w_stage2")
            nc.sync.dma_start(out=w_stage2[:], in_=wl.rearrange("(c p) n -> p c n", p=P))
            nc.scalar.activation(out=wl_sb[:], in_=w_stage2[:], func=AF.Copy)
            bp_stage = cpool.tile([1, P + G * V], F32, name="bp_stage")
            bp_sb = cpool.tile([1, P + G * V], F32R, name="bp_sb")
            nc.sync.dma_start(out=bp_stage[:], in_=bpack[:])
            nc.scalar.activation(out=bp_sb[:], in_=bp_stage[:], func=AF.Copy)
            ones_stage = cpool.tile([P, 1], F32, name="ones_stage")
            ones_sb = cpool.tile([P, 1], F32R, name="ones_sb")
            nc.gpsimd.memset(ones_stage[:], 1.0)
            nc.scalar.activation(out=ones_sb[:], in_=ones_stage[:], func=AF.Copy)

            pp_psum = [
                pppool.tile([1, V], F32, tag=f"pp{g}", name=f"pp_psum{g}")
                for g in range(G)
            ]

            for i in range(ntiles):
                xr_sb = xpool.tile([P, KCH, P], F32R, tag="xr", name="xr_sb")
                xl_sb = xpool.tile([P, KCH, P], F32R, tag="xl", name="xl_sb")
                rsl = slice(i * P, (i + 1) * P)
                nc.sync.dma_start(
                    out=xr_sb[:],
                    in_=xr_t.rearrange("(c p) r -> p c r", p=P)[:, :, rsl].bitcast(F32R),
                )
                nc.sync.dma_start(
                    out=xl_sb[:],
                    in_=xl_t.rearrange("(c p) r -> p c r", p=P)[:, :, rsl].bitcast(F32R),
                )
                q_sb = wkpool.tile([P, G * D], F32, tag="q", name="q_sb")
                for g in range(G):
                    gs = slice(g * V, (g + 1) * V)
                    lg_ps = pspool.tile([P, V], F32, tag=f"lg{g}", name="lg_ps")
                    nc.tensor.matmul(
                        out=lg_ps[:], lhsT=bp_sb[:, :P],
                        rhs=bp_sb[:, P + g * V : P + (g + 1) * V],
                        start=True, stop=False,
                    )
                    for c in range(KCH):
                        nc.tensor.matmul(
                            out=lg_ps[:], lhsT=xr_sb[:, c, :], rhs=wr_sb[:, c, gs],
                            start=False, stop=False,
                        )
                    for c in range(KCH):
                        nc.tensor.matmul(
                            out=lg_ps[:], lhsT=xl_sb[:, c, :], rhs=wr_sb[:, c, gs],
                            start=False, stop=False,
                        )
                    for c in range(KCH):
                        nc.tensor.matmul(
                            out=lg_ps[:], lhsT=xr_sb[:, c, :], rhs=wl_sb[:, c, gs],
                            start=False, stop=(c == KCH - 1),
                        )
                    lg_sb = wkpool.tile([P, V], F32, tag=f"lg_sb{g}", name="lg_sb")
                    nc.scalar.activation(out=lg_sb[:], in_=lg_ps[:], func=AF.Copy)
                    negmax = smpool.tile([P, 1], F32, tag=f"negmax{g}", name="negmax")
                    nc.vector.tensor_reduce(
                        out=negmax[:], in_=lg_sb[:], axis=mybir.AxisListType.X,
                        op=mybir.AluOpType.max, negate=True,
                    )
                    max8 = smpool.tile([P, 8], F32, tag=f"max8{g}", name="max8")
                    nc.vector.max(out=max8[:], in_=lg_sb[:])
                    idx = smpool.tile([P, 8], mybir.dt.uint32, tag=f"idx{g}", name="idx")
                    nc.vector.max_index(out=idx[:], in_max=max8[:], in_values=lg_sb[:])
                    e1 = wkpool.tile([P, V], F32, tag=f"e1{g}", name="e1")
                    s_sb = smpool.tile([P, 1], F32, tag=f"s{g}", name="s_sb")
                    nc.scalar.activation(
                        out=e1[:], in_=lg_ps[:], func=AF.Exp,
                        bias=negmax[:], scale=1.0, accum_out=s_sb[:],
                    )
                    lns = smpool.tile([P, 1], F32, tag=f"lns{g}", name="lns")
                    nc.scalar.activation(out=lns[:], in_=s_sb[:], func=AF.Ln)
                    bias2 = smpool.tile([P, 1], F32, tag=f"b2{g}", name="bias2")
                    nc.vector.tensor_tensor(
                        out=bias2[:], in0=negmax[:], in1=lns[:],
                        op=mybir.AluOpType.subtract,
                    )
                    e2 = wkpool.tile([P, V], F32R, tag=f"e2{g}", name="e2")
                    nc.scalar.activation(
                        out=e2[:], in_=lg_ps[:], func=AF.Exp, bias=bias2[:], scale=1.0,
                    )
                    nc.tensor.matmul(
                        out=pp_psum[g][:], lhsT=ones_sb[:], rhs=e2[:],
                        start=(i == 0), stop=(i == ntiles - 1),
                    )
                    nc.gpsimd.indirect_dma_start(
                        out=q_sb[:, g * D : (g + 1) * D],
                        out_offset=None,
                        in_=cb[:],
                        in_offset=IndirectOffsetOnAxis(ap=idx[:, :1], axis=0),
                        element_offset=g * V * D,
                    )
                nc.sync.dma_start(out=q[rsl, :], in_=q_sb[:])

            for g in range(G):
                pp_sb = smpool.tile([1, V], F32, tag=f"ppsb{g}", name="pp_sb")
                nc.vector.tensor_copy(out=pp_sb[:], in_=pp_psum[g][:])
                nc.sync.dma_start(out=pp[g : g + 1, :], in_=pp_sb[:])

    return nc


@functools.lru_cache(maxsize=1)
def _bass_program():
    import sys
    if "/opt/trn_rl_repo" not in sys.path:
        sys.path.insert(0, "/opt/trn_rl_repo")
    import concourse.bacc as bacc

    nc = bacc.Bacc("TRN2", target_bir_lowering=False, debug=False)
    _build(nc, ROWS)
    nc.compile()
    return nc


def _kernel_bass(x, W, b, cb):
    from concourse.bass_utils import run_bass_kernel_spmd

    nc = _bass_program()

    xr = _round_fp32r(x.reshape(-1, FEAT))
    xl = _round_fp32r(x.reshape(-1, FEAT) - xr)
    wr_ = _round_fp32r(W)
    wl_ = _round_fp32r(W - wr_)
    bp = np.concatenate([np.ones(P, np.float32), _round_fp32r(b)])[None, :]

    xr_s = xr.reshape(N_CORES, ROWS, FEAT)
    xl_s = xl.reshape(N_CORES, ROWS, FEAT)
    in_maps = [
        {
            "xr_t": np.ascontiguousarray(xr_s[c].T),
            "xl_t": np.ascontiguousarray(xl_s[c].T),
            "wr": wr_, "wl": wl_, "bpack": bp, "cb": cb,
        }
        for c in range(N_CORES)
    ]
    res = run_bass_kernel_spmd(nc, in_maps, list(range(N_CORES)))
    q = np.concatenate([res.results[c]["q"] for c in range(N_CORES)], axis=0)
    pp = np.stack([res.results[c]["pp"] for c in range(N_CORES)]).astype(np.float64)
    return q, pp.sum(axis=0)


@functools.lru_cache(maxsize=1)
def _jax_fallback():
    import jax
    import jax.numpy as jnp

    devs = jax.devices()[:N_CORES]

    def shard_fn(x2d, W, b, cb):
        logits = x2d @ W + b
        lg = logits.reshape(-1, GROUPS, NUM_VARS)
        k = jnp.argmax(lg, axis=-1)
        cbg = cb.reshape(GROUPS, NUM_VARS, VAR_DIM)
        qs = jnp.concatenate(
            [jnp.take(cbg[g], k[:, g], axis=0) for g in range(GROUPS)], axis=-1
        )
        m = lg.max(axis=-1, keepdims=True)
        e = jnp.exp(lg - m)
        p = e / e.sum(axis=-1, keepdims=True)
        return qs, p.sum(axis=0)

    return jax.pmap(shard_fn, devices=devs, in_axes=(0, None, None, None))


def _kernel_jax(x, W, b, cb):
    pm = _jax_fallback()
    xs = x.reshape(N_CORES, ROWS, FEAT)
    q_sh, pp_sh = pm(xs, W, b, cb)
    q = np.asarray(q_sh).reshape(-1, GROUPS * VAR_DIM)
    return q, np.asarray(pp_sh, dtype=np.float64).sum(axis=0)


def kernel(x, W, b, codebook):
    x = np.asarray(x, dtype=np.float32)
    W = np.asarray(W, dtype=np.float32)
    b = np.asarray(b, dtype=np.float32)
    cb = np.asarray(codebook, dtype=np.float32).reshape(GROUPS * NUM_VARS, VAR_DIM)
    bsz, tsz, _ = x.shape
    bt = bsz * tsz

    try:
        q2d, pp = _kernel_bass(x, W, b, cb)
    except Exception:
        q2d, pp = _kernel_jax(x, W, b, cb)

    q = q2d.reshape(bsz, tsz, GROUPS * VAR_DIM)
    avg_probs = pp / float(bt)  # [G, V] float64
    ppl = np.exp(-np.sum(avg_probs * np.log(avg_probs + EPS), axis=-1)).sum()
    total = GROUPS * NUM_VARS
    qppl = (total - ppl) / total
    return q, np.float32(qppl), np.float32(CURR_TEMP)
